# revision 7
# baseline (speedup 1.0000x reference)
"""Trainium2 Bass kernel for nn_Compressor (4-layer Perceiver compressor).

Sharding: 8 cores = 4 batch shards x 2 latent-halves (no blocking
collectives on the latent chain).  Core c handles batch c//2 and latent
rows [256*(c%2), 256*(c%2)+256).  Each core also computes K/V for its 8
heads (c%2 half) and the pair (2b, 2b+1) exchanges them with a pairwise
AllGather that runs entirely OFF the critical path (k/v depend only on
the fixed input embeddings, so layer l+1's gather overlaps layer l's
FFN + attention).  Attention/Wo/FFN are then fully local per core.

On-device layout is fully transposed (feature dim on partitions):
  - latT master [d=2048 -> 16 tiles x 128p, n=256] fp32 resident in SBUF
  - xhatT (pre-normalized embeddings, host-computed) streamed per s-chunk
  - kT [dh, seq] and v [seq, dh] staged to DRAM, AllGathered pairwise
  - LN stats via ones-matmuls using var = E[x^2] - mu^2 (single pass)
  - softmax without max-shift (|sim| < ~6), denominator via ones-matmul
    over jt-paired exp tiles [128, 512]
Matmul operands bf16 (LN gains + attention scale folded into weights on
the host); accumulation fp32 in PSUM; residual chain fp32.
"""

import sys
import types

sys.path.insert(0, "/opt/trn_rl_repo")

import numpy as np
import ml_dtypes

BF16 = ml_dtypes.bfloat16

L, DIM, H, DH, FF = 4, 2048, 16, 128, 8192
INNER = H * DH
EPS = 1e-5
B, NLAT, S = 4, 512, 2048
TP = 2
NL = NLAT // TP        # 256 latent rows per core
HPC = H // TP          # 8 kv heads per core
CKV = HPC * DH         # 1024 kv cols per core
NCORES = 8
DT = DIM // 128        # 16 d-tiles
FT = FF // 128         # 64 ffn tiles (full FF per core)
NG = HPC // 2          # 4 kv head groups of 2
SC = 4                 # 4 seq chunks of 512

TRACE = False          # test.py can flip this for profiling

_cache = {}


def _install_ntff_shim():
    """antenv.axon_hooks is absent in this image; provide it so trace=True works."""
    try:
        import antenv
        if "antenv.axon_hooks" in sys.modules:
            return
        hooks = types.ModuleType("antenv.axon_hooks")
        _h = [None]
        hooks.set_axon_ntff_profile_hook = lambda h: _h.__setitem__(0, h)
        hooks.get_axon_ntff_profile_hook = lambda: _h[0]
        sys.modules["antenv.axon_hooks"] = hooks
        antenv.axon_hooks = hooks
        from trn_agent_boot.trn_boot import _ntff_profile_via_ctypes
        hk = _ntff_profile_via_ctypes("/opt/axon/libaxon_pjrt.so")
        if hk is not None:
            hooks.set_axon_ntff_profile_hook(hk)
    except Exception:
        pass


def _build(with_bias):
    """Build the SPMD Bass program (same for every core)."""
    import concourse.bass as bass
    import concourse.tile as tile
    import concourse.mybir as mybir
    from concourse import bacc

    f32 = mybir.dt.float32
    bf16 = mybir.dt.bfloat16

    nc = bacc.Bacc("TRN2", target_bir_lowering=False, debug=False,
                   num_devices=NCORES)

    # ---- DRAM parameters (per-core shards; SPMD-identical shapes) ----
    d_xhat = nc.dram_tensor("xhat", [SC, 128, DT, 512], bf16, kind="ExternalInput").ap()
    d_lat0 = nc.dram_tensor("lat0", [128, DT, NL], f32, kind="ExternalInput").ap()
    d_wq = nc.dram_tensor("wq", [L, H, 128, DT, 128], bf16, kind="ExternalInput").ap()
    d_wk = nc.dram_tensor("wk", [L, NG, 128, DT, 256], bf16, kind="ExternalInput").ap()
    d_wv = nc.dram_tensor("wv", [L, NG, 128, DT, 256], bf16, kind="ExternalInput").ap()
    d_wo = nc.dram_tensor("wo", [L, DT, 128, H, 128], bf16, kind="ExternalInput").ap()
    d_w1 = nc.dram_tensor("w1", [L, FT, 128, DT, 128], bf16, kind="ExternalInput").ap()
    d_w2 = nc.dram_tensor("w2", [L, DT, 128, FT, 128], bf16, kind="ExternalInput").ap()
    d_fng = nc.dram_tensor("fng", [128, DT], f32, kind="ExternalInput").ap()
    d_fnb = nc.dram_tensor("fnb", [128, DT], f32, kind="ExternalInput").ap()
    d_bq = d_bk = d_b1 = d_bv = None
    if with_bias:
        d_bq = nc.dram_tensor("bq", [L, 128, H], f32, kind="ExternalInput").ap()
        d_bk = nc.dram_tensor("bk", [L, 128, HPC], f32, kind="ExternalInput").ap()
        d_b1 = nc.dram_tensor("b1", [L, 128, FT], f32, kind="ExternalInput").ap()
        d_bv = nc.dram_tensor("bv", [L, NG, 128, 256], f32, kind="ExternalInput").ap()
    d_out = nc.dram_tensor("outT", [128, DT, NL], f32, kind="ExternalOutput").ap()

    RG = [[0, 1], [2, 3], [4, 5], [6, 7]]

    from contextlib import ExitStack

    with tile.TileContext(nc) as tc:
        with ExitStack() as stack:
            def pool(name, bufs, space=None):
                kw = {"space": space} if space else {}
                return stack.enter_context(
                    tc.tile_pool(name=name, bufs=bufs, **kw))

            pC = pool("pC", 1)
            pLat = pool("pLat", 1)
            pHat = pool("pHat", 1)
            pQ = pool("pQ", 1)
            pO = pool("pO", 1)
            pA = pool("pA", 1)
            pXs = pool("pXs", 2)
            pWk = pool("pWk", 2)
            pWv = pool("pWv", 2)
            pKs = pool("pKs", 3)
            pVs = pool("pVs", 3)
            pKh = pool("pKh", 2)
            pVh = pool("pVh", 2)
            pEx = pool("pEx", 3)
            pW = pool("pW", 3)
            pW2 = pool("pW2", 2)
            pSm = pool("pSm", 6)
            pSq = pool("pSq", 4)
            pStg = pool("pStg", 4)
            psA = pool("psA", 2, "PSUM")
            psB = pool("psB", 2, "PSUM")
            psC = pool("psC", 4, "PSUM")
            pDram = pool("pDram", 8, "DRAM")

            Act = mybir.ActivationFunctionType
            Alu = mybir.AluOpType

            # ---- constants / whole-run residents ----
            ones_b = pC.tile([128, 128], bf16, tag="onesb")
            nc.vector.memset(ones_b, 1.0)
            eps_sb = pC.tile([128, 1], f32, tag="eps")
            nc.vector.memset(eps_sb, EPS)
            fng_sb = pC.tile([128, DT], f32, tag="fng")
            nc.sync.dma_start(fng_sb[:], d_fng)
            fnb_sb = pC.tile([128, DT], f32, tag="fnb")
            nc.sync.dma_start(fnb_sb[:], d_fnb)
            if with_bias:
                bq_sb = pC.tile([128, L, H], f32, tag="bq")
                nc.sync.dma_start(bq_sb[:], d_bq.rearrange("l p h -> p l h"))
                bk_sb = pC.tile([128, L, HPC], f32, tag="bk")
                nc.sync.dma_start(bk_sb[:], d_bk.rearrange("l p h -> p l h"))
                b1_sb = pC.tile([128, L, FT], f32, tag="b1")
                nc.sync.dma_start(b1_sb[:], d_b1.rearrange("l p h -> p l h"))

            latT = pLat.tile([128, DT, NL], f32, tag="lat")
            nc.sync.dma_start(latT[:], d_lat0)

            def layernorm_hat():
                """LN on latT -> fresh lat_hat (bf16) in pHat; no gain/bias
                (folded into downstream weights).  var = E[x^2] - mu^2."""
                mu_ps = psC.tile([128, NL], f32, tag="c")
                x2_ps = psC.tile([128, NL], f32, tag="c")
                for dt in range(DT):
                    lb = pSq.tile([128, NL], bf16, tag="sq")
                    nc.vector.tensor_copy(lb[:], latT[:, dt, :])
                    nc.tensor.matmul(mu_ps[:], ones_b[:], lb[:],
                                     start=(dt == 0), stop=(dt == DT - 1))
                    sq = pSq.tile([128, NL], bf16, tag="sq")
                    nc.vector.tensor_mul(sq[:], lb[:], lb[:])
                    nc.tensor.matmul(x2_ps[:], ones_b[:], sq[:],
                                     start=(dt == 0), stop=(dt == DT - 1))
                mu = pSm.tile([128, NL], f32, tag="sm")
                nc.scalar.activation(mu[:], mu_ps[:], Act.Copy, scale=1.0 / DIM)
                ex2 = pSm.tile([128, NL], f32, tag="sm")
                nc.scalar.activation(ex2[:], x2_ps[:], Act.Copy, scale=1.0 / DIM)
                var = pSm.tile([128, NL], f32, tag="sm")
                nc.vector.tensor_mul(var[:], mu[:], mu[:])
                nc.vector.tensor_sub(var[:], ex2[:], var[:])
                sd = pSm.tile([128, NL], f32, tag="sm")
                nc.scalar.activation(sd[:], var[:], Act.Sqrt, bias=eps_sb[:])
                rstd = pSm.tile([128, NL], f32, tag="sm")
                nc.vector.reciprocal(rstd[:], sd[:])
                hat = pHat.tile([128, DT, NL], bf16, tag="hat")
                for dt in range(DT):
                    t1 = pStg.tile([128, NL], f32, tag="stg")
                    nc.vector.tensor_sub(t1[:], latT[:, dt, :], mu[:])
                    nc.vector.tensor_mul(hat[:, dt, :], t1[:], rstd[:])
                return hat

            def kv_group(l, g):
                """Project k (2 heads) and v for head group g of layer l,
                stage to DRAM and AllGather with the pair partner.
                Independent of the latents chain."""
                wk_t = pWk.tile([128, DT, 256], bf16, tag="wk")
                nc.sync.dma_start(wk_t[:], d_wk[l, g])
                wv_t = pWv.tile([128, DT, 256], bf16, tag="wv")
                nc.sync.dma_start(wv_t[:], d_wv[l, g])
                agin = pDram.tile([2, 2, 16, 128, 128], bf16, tag="agin")
                agout = pDram.tile([2, 2, 2, 16, 128, 128], bf16, tag="agout")
                if with_bias:
                    bvt = pVs.tile([128, 256], f32, tag="bv")
                    nc.sync.dma_start(bvt[:], d_bv[l, g])
                for sc in range(SC):
                    xh = pXs.tile([128, DT, 512], bf16, tag="xh")
                    nc.sync.dma_start(xh[:], d_xhat[sc])
                    for hl in range(2):
                        kp = psA.tile([128, 512], f32, tag="a")
                        for dt in range(DT):
                            nc.tensor.matmul(
                                kp[:], wk_t[:, dt, hl * 128:(hl + 1) * 128],
                                xh[:, dt, :],
                                start=(dt == 0), stop=(dt == DT - 1))
                        k_sb = pKs.tile([128, 512], bf16, tag="ks")
                        if with_bias:
                            nc.scalar.activation(
                                k_sb[:], kp[:], Act.Identity,
                                bias=bk_sb[:, l, 2 * g + hl:2 * g + hl + 1])
                        else:
                            nc.vector.tensor_copy(k_sb[:], kp[:])
                        nc.sync.dma_start(
                            agin[hl, 0, sc * 4:(sc + 1) * 4].rearrange(
                                "t p s -> p t s"),
                            k_sb[:].rearrange("p (t s) -> p t s", t=4))
                    for st in range(4):
                        vp = psA.tile([128, 512], f32, tag="a")
                        for dt in range(DT):
                            nc.tensor.matmul(
                                vp[:, :256],
                                xh[:, dt, st * 128:(st + 1) * 128],
                                wv_t[:, dt, :],
                                start=(dt == 0), stop=(dt == DT - 1))
                        v_sb = pVs.tile([128, 256], bf16, tag="vs")
                        if with_bias:
                            nc.vector.tensor_add(v_sb[:], vp[:, :256], bvt[:])
                        else:
                            nc.vector.tensor_copy(v_sb[:], vp[:, :256])
                        nc.sync.dma_start(
                            agin[:, 1, sc * 4 + st].rearrange("h p d -> p h d"),
                            v_sb[:].rearrange("p (h d) -> p h d", h=2))
                nc.gpsimd.collective_compute(
                    "AllGather", Alu.bypass, replica_groups=RG,
                    ins=[agin[:].opt()], outs=[agout[:].opt()])
                return agout

            pending = {}
            for g in range(NG):
                pending[(0, g)] = kv_group(0, g)

            for l in range(L):
                # ---------- LN over latents + Q projection (all 16 heads) ----------
                hat = layernorm_hat()
                q_all = pQ.tile([128, H, NL], bf16, tag="q")
                for h in range(H):
                    wq_t = pW.tile([128, DT, 128], bf16, tag="w")
                    nc.sync.dma_start(wq_t[:], d_wq[l, h])
                    qp = psC.tile([128, NL], f32, tag="c")
                    for dt in range(DT):
                        nc.tensor.matmul(qp[:], wq_t[:, dt, :], hat[:, dt, :],
                                         start=(dt == 0), stop=(dt == DT - 1))
                    if with_bias:
                        nc.scalar.activation(q_all[:, h, :], qp[:], Act.Identity,
                                             bias=bq_sb[:, l, h:h + 1])
                    else:
                        nc.vector.tensor_copy(q_all[:, h, :], qp[:])

                # ---------- attention (16 heads, 256 latents) ----------
                o_all = pO.tile([128, H, NL], bf16, tag="o")
                for g in range(NG):
                    agout = pending.pop((l, g))
                    for r in range(2):
                        for hl in range(2):
                            h = r * HPC + 2 * g + hl
                            k_h = pKh.tile([128, 16, 128], bf16, tag="kh")
                            nc.sync.dma_start(
                                k_h[:], agout[r, hl, 0].rearrange("t p s -> p t s"))
                            v_h = pVh.tile([128, 16, 128], bf16, tag="vh")
                            nc.sync.dma_start(
                                v_h[:], agout[r, hl, 1].rearrange("t p d -> p t d"))
                            den_ps = psC.tile([128, 512], f32, tag="c")
                            o_ps = psC.tile([128, NL], f32, tag="c")
                            for jp in range(8):
                                sp = psB.tile([128, 512], f32, tag="sim")
                                nc.tensor.matmul(sp[:, :256], k_h[:, 2 * jp, :],
                                                 q_all[:, h, :], start=True, stop=True)
                                nc.tensor.matmul(sp[:, 256:], k_h[:, 2 * jp + 1, :],
                                                 q_all[:, h, :], start=True, stop=True)
                                ex = pEx.tile([128, 512], bf16, tag="ex")
                                nc.scalar.activation(ex[:], sp[:], Act.Exp)
                                nc.tensor.matmul(den_ps[:], ones_b[:], ex[:],
                                                 start=(jp == 0), stop=(jp == 7))
                                nc.tensor.matmul(o_ps[:], v_h[:, 2 * jp, :],
                                                 ex[:, :256],
                                                 start=(jp == 0), stop=False)
                                nc.tensor.matmul(o_ps[:], v_h[:, 2 * jp + 1, :],
                                                 ex[:, 256:],
                                                 start=False, stop=(jp == 7))
                            dh0 = pSm.tile([128, NL], f32, tag="sm")
                            nc.vector.tensor_copy(dh0[:], den_ps[:, :256])
                            den = pSm.tile([128, NL], f32, tag="sm")
                            nc.vector.tensor_add(den[:], dh0[:], den_ps[:, 256:])
                            rec = pSm.tile([128, NL], f32, tag="sm")
                            nc.vector.reciprocal(rec[:], den[:])
                            nc.vector.tensor_mul(o_all[:, h, :], o_ps[:], rec[:])

                # ---------- attention out projection + residual ----------
                for dt in range(DT):
                    wo_t = pW.tile([128, H, 128], bf16, tag="w")
                    nc.sync.dma_start(wo_t[:], d_wo[l, dt])
                    yp = psA.tile([128, NL], f32, tag="a")
                    for ct in range(H):
                        nc.tensor.matmul(yp[:], wo_t[:, ct, :], o_all[:, ct, :],
                                         start=(ct == 0), stop=(ct == H - 1))
                    nc.vector.tensor_add(latT[:, dt, :], latT[:, dt, :], yp[:])

                # ---------- next layer's k/v (fills AllGather slack) ----------
                if l + 1 < L:
                    for g in range(NG):
                        pending[(l + 1, g)] = kv_group(l + 1, g)

                # ---------- FFN ----------
                hat2 = layernorm_hat()
                a_all = pA.tile([128, FT // 2, 512], bf16, tag="aa")
                for fp in range(FT // 2):
                    hp = psA.tile([128, 512], f32, tag="a")
                    for fh in range(2):
                        ft = 2 * fp + fh
                        w1_t = pW.tile([128, DT, 128], bf16, tag="w")
                        nc.sync.dma_start(w1_t[:], d_w1[l, ft])
                        for dt in range(DT):
                            nc.tensor.matmul(hp[:, fh * 256:(fh + 1) * 256],
                                             w1_t[:, dt, :], hat2[:, dt, :],
                                             start=(dt == 0), stop=(dt == DT - 1))
                    if with_bias:
                        for fh in range(2):
                            ft = 2 * fp + fh
                            nc.scalar.activation(
                                a_all[:, fp, fh * 256:(fh + 1) * 256],
                                hp[:, fh * 256:(fh + 1) * 256], Act.Silu,
                                bias=b1_sb[:, l, ft:ft + 1])
                    else:
                        nc.scalar.activation(a_all[:, fp, :], hp[:], Act.Silu)
                for dt in range(DT):
                    yp = psA.tile([128, NL], f32, tag="a")
                    for half in range(2):
                        w2_t = pW2.tile([128, FT // 2, 128], bf16, tag="w2")
                        nc.sync.dma_start(
                            w2_t[:], d_w2[l, dt, :, half * 32:(half + 1) * 32, :])
                        for fh in range(FT // 2):
                            ft = half * 32 + fh
                            nc.tensor.matmul(
                                yp[:], w2_t[:, fh, :],
                                a_all[:, ft // 2, (ft % 2) * 256:(ft % 2) * 256 + 256],
                                start=(ft == 0), stop=(ft == FT - 1))
                    nc.vector.tensor_add(latT[:, dt, :], latT[:, dt, :], yp[:])

            # ---------- final layernorm (with gain/bias) ----------
            mu_ps = psC.tile([128, NL], f32, tag="c")
            x2_ps = psC.tile([128, NL], f32, tag="c")
            for dt in range(DT):
                lb = pSq.tile([128, NL], bf16, tag="sq")
                nc.vector.tensor_copy(lb[:], latT[:, dt, :])
                nc.tensor.matmul(mu_ps[:], ones_b[:], lb[:],
                                 start=(dt == 0), stop=(dt == DT - 1))
                sq = pSq.tile([128, NL], bf16, tag="sq")
                nc.vector.tensor_mul(sq[:], lb[:], lb[:])
                nc.tensor.matmul(x2_ps[:], ones_b[:], sq[:],
                                 start=(dt == 0), stop=(dt == DT - 1))
            mu = pSm.tile([128, NL], f32, tag="sm")
            nc.scalar.activation(mu[:], mu_ps[:], Act.Copy, scale=1.0 / DIM)
            ex2 = pSm.tile([128, NL], f32, tag="sm")
            nc.scalar.activation(ex2[:], x2_ps[:], Act.Copy, scale=1.0 / DIM)
            var = pSm.tile([128, NL], f32, tag="sm")
            nc.vector.tensor_mul(var[:], mu[:], mu[:])
            nc.vector.tensor_sub(var[:], ex2[:], var[:])
            sd = pSm.tile([128, NL], f32, tag="sm")
            nc.scalar.activation(sd[:], var[:], Act.Sqrt, bias=eps_sb[:])
            rstd = pSm.tile([128, NL], f32, tag="sm")
            nc.vector.reciprocal(rstd[:], sd[:])
            for dt in range(DT):
                t1 = pStg.tile([128, NL], f32, tag="stg")
                nc.vector.tensor_sub(t1[:], latT[:, dt, :], mu[:])
                t2 = pStg.tile([128, NL], f32, tag="stg")
                nc.vector.tensor_mul(t2[:], t1[:], rstd[:])
                t3 = pStg.tile([128, NL], f32, tag="stg")
                nc.scalar.activation(t3[:], t2[:], Act.Identity,
                                     scale=fng_sb[:, dt:dt + 1],
                                     bias=fnb_sb[:, dt:dt + 1])
                nc.sync.dma_start(d_out[:, dt, :], t3[:])

    nc.compile()
    return nc


def _tile_kxm(w, kt, mt):
    """[K, M] -> [M//128 blocks][128p(K-sub), K//128, 128(M)] host layout."""
    K, M = w.shape
    return np.ascontiguousarray(
        w.reshape(K // 128, 128, M // 128, 128).transpose(2, 1, 0, 3))


def kernel(**inputs):
    inp = {k: np.asarray(v) for k, v in inputs.items()}
    latents = inp["latents"].astype(np.float32)
    seg = inp["seg_embeddings"].astype(np.float32)
    pos = inp["pos_emb"].astype(np.float32)
    nx_g, nx_b = inp["nx_g"].astype(np.float32), inp["nx_b"].astype(np.float32)
    nl_g, nl_b = inp["nl_g"].astype(np.float32), inp["nl_b"].astype(np.float32)
    Wq, Wkv, Wo = (inp["Wq"].astype(np.float32), inp["Wkv"].astype(np.float32),
                   inp["Wo"].astype(np.float32))
    fln_g, fln_b = inp["fln_g"].astype(np.float32), inp["fln_b"].astype(np.float32)
    W1, W2 = inp["W1"].astype(np.float32), inp["W2"].astype(np.float32)
    fn_g, fn_b = inp["fn_g"].astype(np.float32), inp["fn_b"].astype(np.float32)

    scale = DH ** -0.5

    # ---- host prep: normalized embeddings (input-only, layer-independent) ----
    emb = seg + pos[None, :S, :]                       # [B, S, D]
    mu = emb.mean(-1, keepdims=True)
    var = ((emb - mu) ** 2).mean(-1, keepdims=True)
    xhat = (emb - mu) / np.sqrt(var + EPS)             # [B, S, D]

    # per-core shards -------------------------------------------------------
    xhat_core = []                                     # per batch: [4,128,DT,512] bf16
    for b in range(B):
        xT = np.ascontiguousarray(xhat[b].T)           # [D, S]
        xt = xT.reshape(DT, 128, SC, 512).transpose(2, 1, 0, 3)
        xhat_core.append(np.ascontiguousarray(xt.astype(BF16)))
    lat_core = {}                                      # (b, t) -> [128, DT, NL]
    for b in range(B):
        lT = np.ascontiguousarray(latents[b].T)        # [D, NLAT]
        for t in range(TP):
            lh = lT[:, t * NL:(t + 1) * NL]
            lat_core[(b, t)] = np.ascontiguousarray(
                lh.reshape(DT, 128, NL).transpose(1, 0, 2)).astype(np.float32)

    # shared (full) weights -------------------------------------------------
    wq_l, wo_l, w1_l, w2_l = [], [], [], []
    bq_l, b1_l = [], []
    for l in range(L):
        wq_eff = (nl_g[l][:, None] * Wq[l]) * scale
        wq_l.append(_tile_kxm(wq_eff, DT, H).astype(BF16))
        bq = (nl_b[l] @ Wq[l]) * scale
        bq_l.append(np.ascontiguousarray(bq.reshape(H, 128).T))
        wo_t = Wo[l].reshape(H, 128, DT, 128).transpose(2, 1, 0, 3)
        wo_l.append(np.ascontiguousarray(wo_t).astype(BF16))
        w1_eff = fln_g[l][:, None] * W1[l]
        w1_l.append(_tile_kxm(w1_eff, DT, FT).astype(BF16))
        b1 = fln_b[l] @ W1[l]
        b1_l.append(np.ascontiguousarray(b1.reshape(FT, 128).T))
        w2_t = W2[l].reshape(FT, 128, DT, 128).transpose(2, 1, 0, 3)
        w2_l.append(np.ascontiguousarray(w2_t).astype(BF16))
    wq_all = np.stack(wq_l)
    wo_all = np.stack(wo_l)
    w1_all = np.stack(w1_l)
    w2_all = np.stack(w2_l)
    bq_all = np.stack(bq_l).astype(np.float32)
    b1_all = np.stack(b1_l).astype(np.float32)

    # per-TP-half kv weights ------------------------------------------------
    whalf = []
    for t in range(TP):
        c0 = t * CKV
        wk_l, wv_l, bk_l, bv_l = [], [], [], []
        for l in range(L):
            wk_eff = nx_g[l][:, None] * Wkv[l][:, c0:c0 + CKV]
            wv_eff = nx_g[l][:, None] * Wkv[l][:, INNER + c0:INNER + c0 + CKV]
            bk = nx_b[l] @ Wkv[l][:, c0:c0 + CKV]
            bv = nx_b[l] @ Wkv[l][:, INNER + c0:INNER + c0 + CKV]
            wk_t = wk_eff.reshape(DT, 128, NG, 256).transpose(2, 1, 0, 3)
            wv_t = wv_eff.reshape(DT, 128, NG, 256).transpose(2, 1, 0, 3)
            wk_l.append(np.ascontiguousarray(wk_t).astype(BF16))
            wv_l.append(np.ascontiguousarray(wv_t).astype(BF16))
            bk_l.append(np.ascontiguousarray(bk.reshape(HPC, 128).T))
            bv_l.append(np.ascontiguousarray(
                np.broadcast_to(bv.reshape(NG, 1, 256), (NG, 128, 256)).copy()))
        whalf.append(dict(
            wk=np.stack(wk_l), wv=np.stack(wv_l),
            bk=np.stack(bk_l).astype(np.float32),
            bv=np.stack(bv_l).astype(np.float32)))

    fng = np.ascontiguousarray(fn_g.reshape(DT, 128).T).astype(np.float32)
    fnb = np.ascontiguousarray(fn_b.reshape(DT, 128).T).astype(np.float32)

    with_bias = bool(np.any(nx_b != 0.0) or np.any(nl_b != 0.0)
                     or np.any(fln_b != 0.0))

    _install_ntff_shim()

    key = ("nc", with_bias)
    if key not in _cache:
        _cache[key] = _build(with_bias)
    nc = _cache[key]

    in_maps = []
    for c in range(NCORES):
        b, t = c // 2, c % 2
        w = whalf[t]
        m = dict(xhat=xhat_core[b], lat0=lat_core[(b, t)],
                 wq=wq_all, wk=w["wk"], wv=w["wv"], wo=wo_all,
                 w1=w1_all, w2=w2_all, fng=fng, fnb=fnb)
        if with_bias:
            m["bq"] = bq_all
            m["bk"] = w["bk"]
            m["b1"] = b1_all
            m["bv"] = w["bv"]
        in_maps.append(m)

    from concourse.bass_utils import run_bass_kernel_spmd
    res = run_bass_kernel_spmd(nc, in_maps, list(range(NCORES)), trace=TRACE)
    if TRACE:
        kernel.last_exec_time_ns = res.exec_time_ns
        kernel.last_profile = res.profile_json

    outs = []
    for b in range(B):
        halves = []
        for t in range(TP):
            o = res.results[2 * b + t]["outT"]          # [128, DT, NL]
            outT = o.transpose(1, 0, 2).reshape(DIM, NL)  # [D, NL]
            halves.append(outT.T)                         # [NL, D]
        outs.append(np.concatenate(halves, axis=0))       # [NLAT, D]
    return np.stack(outs).astype(np.float32)
